# revision 1
# baseline (speedup 1.0000x reference)
"""Trainium2 Bass kernel for Mistral-style sliding-window GQA attention.

Problem (hardcoded shapes):
  hidden_states [2048, 4096] f32, Wq [4096, 4096], Wk/Wv [4096, 1024],
  Wo [4096, 4096], cu_seqlens [3] int32. 32 Q heads / 8 KV heads,
  head_dim 128, sliding window 512, rope theta 10000.

Sharding: tensor-parallel over heads across 8 cores. Core c owns Q heads
[4c, 4c+4) and KV head c (GQA groups align: qh//4 == c). Wq/Wk/Wv are
column-sharded, Wo row-sharded; each core emits a partial [2048, 4096]
output which the host sums.

Device kernel layout choices (per core):
  - hT = hidden^T [4096, 2048] bf16 is the streamed rhs for all
    projections (qT/kT/vT come out in [head_dim, T] layout with weight
    tiles as the stationary operand in natural layout).
  - RoPE: rotate_half is two partition-shifted DVE multiplies against a
    sign-folded sin table; no PE work, no extra permutation matrix.
  - scores are computed transposed (ST[k,q] = kT.T @ qT) for two heads
    at once (q tiles of the head pair interleaved in SBUF), so score
    matmul N=256 and one exp per pair. Softmax skips max-subtraction
    (scores are O(10), far from fp32 exp overflow); the denominator
    comes free as a ones-column appended to V.
  - partial-tile masks (causal diagonal / window edge / arbitrary
    cu_seqlens boundaries) are host-computed 0/1 bf16 tiles (duplicated
    per head pair), applied multiplicatively after exp on GpSimd.
  - attention output [q, dim] is normalized per-partition (reciprocal of
    the ones-column), transposed on the PE into a shared PSUM bank, and
    fed as lhsT to the row-parallel Wo matmul; partials bounce
    PSUM->SBUF (DVE/ACT alternating) and stream to DRAM.
"""

import numpy as np
import ml_dtypes

import concourse.bass as bass
import concourse.tile as tile
from concourse import bacc, mybir
from concourse import bass_utils

# ---- problem constants -------------------------------------------------
T = 2048
HID = 4096
NUM_HEADS = 32
NUM_KV_HEADS = 8
D = 128  # head dim
WINDOW = 512
ROPE_THETA = 10000.0
N_CORES = 8
HPC = NUM_HEADS // N_CORES  # 4 q heads per core
QD = HPC * D  # 512 q-proj cols per core

NT = T // 128  # 16 token tiles
NKT = HID // 128  # 32 hidden k-tiles
NSTRIP = T // 512  # 4 token strips of 512
NOUT = HID // 512  # 8 output column slices

F32 = mybir.dt.float32
BF16 = mybir.dt.bfloat16
SCALE = 1.0 / np.sqrt(D)

_cache = {}


def _host_prep(cu_seqlens):
    """Everything derived from cu_seqlens: positions, rope tables,
    per-tile job list and mask tiles (ST layout [k, q], head-pair
    duplicated to [128, 256])."""
    cu = np.asarray(cu_seqlens, dtype=np.int64)
    tok = np.arange(T)
    seg = np.searchsorted(cu[1:], tok, side="right")
    pos = tok - cu[np.minimum(seg, len(cu) - 1)]

    same = seg[:, None] == seg[None, :]
    causal = pos[None, :] <= pos[:, None]
    win = pos[None, :] >= pos[:, None] - (WINDOW - 1)
    allowed = same & causal & win  # [q, k]

    jobs = []  # jobs[i] = [(j, mask_id | None), ...]
    masks = []
    mask_index = {}
    for i in range(NT):
        row = []
        for j in range(NT):
            blk = allowed[128 * i : 128 * (i + 1), 128 * j : 128 * (j + 1)]
            if not blk.any():
                continue
            if blk.all():
                row.append((j, None))
            else:
                key = blk.tobytes()
                if key not in mask_index:
                    mask_index[key] = len(masks)
                    masks.append(blk.T.astype(np.float32))  # ST layout
                row.append((j, mask_index[key]))
        jobs.append(row)
    if not masks:
        masks.append(np.ones((128, 128), np.float32))
    m = np.stack(masks)
    masks_np = np.concatenate([m, m], axis=2).astype(ml_dtypes.bfloat16)

    inv = 1.0 / (ROPE_THETA ** (np.arange(0, D, 2, dtype=np.float64) / D))
    freqs = pos[:, None].astype(np.float64) * inv[None, :]  # [T, 64]
    emb = np.concatenate([freqs, freqs], axis=1)  # [T, 128]
    cos_t = np.cos(emb).T.astype(np.float32).copy()  # [128, T]
    sin_t = np.sin(emb).T.astype(np.float32)
    # sign-folded: rope(x)[d] = x[d]*cos[d] + x[(d+64)%128] * sin_s[d]
    sin_s = np.concatenate([-sin_t[:64], sin_t[64:]], axis=0).copy()
    ident = np.eye(128, dtype=ml_dtypes.bfloat16)

    return jobs, masks_np, cos_t, sin_s, ident


def _build(jobs, n_masks):
    """Trace the Bass/Tile program (identical on all cores)."""
    nc = bacc.Bacc("TRN2", target_bir_lowering=False, debug=False,
                   num_devices=N_CORES)

    # DRAM I/O (per-core shapes)
    ht_d = nc.dram_tensor("ht", [NSTRIP, NKT // 4, 128, 2048], BF16,
                          kind="ExternalInput").ap()
    wq_d = nc.dram_tensor("wq", [HPC, 128, HID], BF16,
                          kind="ExternalInput").ap()
    wk_d = nc.dram_tensor("wk", [128, HID], BF16, kind="ExternalInput").ap()
    wv_d = nc.dram_tensor("wv", [128, HID], BF16, kind="ExternalInput").ap()
    wo_d = nc.dram_tensor("wo", [HPC, 128, HID], BF16,
                          kind="ExternalInput").ap()
    cos_d = nc.dram_tensor("cos_t", [128, T], F32, kind="ExternalInput").ap()
    sin_d = nc.dram_tensor("sin_s", [128, T], F32, kind="ExternalInput").ap()
    ident_d = nc.dram_tensor("ident", [128, 128], BF16,
                             kind="ExternalInput").ap()
    masks_d = nc.dram_tensor("masks", [n_masks, 128, 256], BF16,
                             kind="ExternalInput").ap()
    out_d = nc.dram_tensor("out", [T, HID], F32, kind="ExternalOutput").ap()

    with tile.TileContext(nc) as tc:
        with tc.tile_pool(name="persist", bufs=1) as pp:
            # resident weights / tables
            wq_sb = [pp.tile([128, HID], BF16, name=f"wq{h}") for h in range(HPC)]
            wk_sb = pp.tile([128, HID], BF16, name="wk_sb")
            wv_sb = pp.tile([128, HID], BF16, name="wv_sb")
            wo_sb = [pp.tile([128, HID], BF16, name=f"wo{h}") for h in range(HPC)]
            cos_sb = pp.tile([128, T], F32, name="cos_sb")
            sin_sb = pp.tile([128, T], F32, name="sin_sb")
            ident_sb = pp.tile([128, 128], BF16, name="ident_sb")
            mask_sb = [pp.tile([128, 256], BF16, name=f"mask{m}")
                       for m in range(n_masks)]
            # activations produced by phase 1, consumed by phase 2
            # qt pairs: [128, 2*T]; cols [256*i + 128*m : +128] = head
            # (2*hp + m), token tile i.
            qt_sb = [pp.tile([128, 2 * T], BF16, name=f"qtp{hp}")
                     for hp in range(2)]
            kt_sb = pp.tile([128, T], BF16, name="kt_sb")
            vaug_sb = [pp.tile([128, D + 1], BF16, name=f"vaug{t}")
                       for t in range(NT)]

            qt_4d = [q.rearrange("p (i m c) -> p i m c", m=2, c=128)
                     for q in qt_sb]

            for t in range(NT):
                nc.vector.memset(vaug_sb[t][:, D : D + 1], 1.0)

            # ---------------- phase 1: projections + RoPE ----------------
            with (
                tc.tile_pool(name="ht_pool", bufs=6) as htp,
                tc.tile_pool(name="rope_tmp", bufs=4) as rtp,
                tc.tile_pool(name="proj_psum", bufs=6, space="PSUM") as ppp,
                tc.tile_pool(name="util_psum", bufs=2, space="PSUM") as upp,
            ):
                def rope(s, h, src):
                    """src: fp32 PSUM [128, 512] pre-rope projection."""
                    ssl = bass.ts(s, 512)
                    if h < HPC:
                        dst = qt_4d[h // 2][:, 4 * s : 4 * s + 4, h % 2, :]
                    else:
                        dst = kt_sb[:, ssl]
                    raw = rtp.tile([128, 512], F32, tag="raw",
                                   name=f"raw{s}_{h}")
                    nc.scalar.copy(raw[:], src[:])
                    t1 = rtp.tile([128, 512], F32, tag="t1",
                                  name=f"t1_{s}_{h}")
                    nc.gpsimd.tensor_mul(t1[:], raw[:], cos_sb[:, ssl])
                    # rotate_half: walrus requires TT operands to share a
                    # start partition, so swap halves via gpsimd copies first
                    # (partition-shifted copies are legal; signs live in sin_s)
                    sw = rtp.tile([128, 512], F32, tag="sw",
                                  name=f"sw{s}_{h}")
                    nc.vector.tensor_scalar_mul(sw[0:64, :],
                                                raw[64:128, :], 1.0)
                    nc.vector.tensor_scalar_mul(sw[64:128, :],
                                                raw[0:64, :], 1.0)
                    t2 = rtp.tile([128, 512], F32, tag="t2",
                                  name=f"t2_{s}_{h}")
                    nc.vector.tensor_mul(t2[:], sw[:], sin_sb[:, ssl])
                    if h < HPC:
                        t1v = t1.rearrange("p (i c) -> p i c", c=128)
                        t2v = t2.rearrange("p (i c) -> p i c", c=128)
                    else:
                        t1v, t2v = t1[:], t2[:]
                    nc.vector.tensor_add(dst, t1v, t2v)

                def v_pipeline(s, ps_v):
                    """ps_v: vT strip PSUM -> 4 v_aug tiles [k, dim]."""
                    vts = rtp.tile([128, 512], BF16, tag="vts", name=f"vts{s}")
                    nc.vector.tensor_copy(vts[:], ps_v[:])
                    vtp = upp.tile([128, 512], BF16, tag="util", name=f"vtp{s}")
                    for tt in range(4):
                        tsl = bass.ts(tt, 128)
                        nc.tensor.transpose(vtp[:, tsl], vts[:, tsl],
                                            ident_sb[:])
                        nc.vector.tensor_copy(vaug_sb[4 * s + tt][:, 0:D],
                                              vtp[:, tsl])

                def proj_round(s, heads, preamble=None, postamble=None):
                    """One k-loop computing projections `heads` (0..3 = q,
                    4 = k, 5 = v) for strip s into len(heads) PSUM banks."""
                    ps = [ppp.tile([128, 512], F32, tag="proj",
                                   name=f"ps{s}_{h}") for h in heads]
                    wt = {4: wk_sb, 5: wv_sb}
                    for g in range(NKT // 4):
                        if preamble is not None:
                            preamble(4 * g)
                        # one DMA carries 4 hidden k-tiles side by side
                        ht_t = htp.tile([128, 2048], BF16, tag="ht",
                                        name=f"ht{s}_{g}_{heads[0]}")
                        nc.sync.dma_start(ht_t[:], ht_d[s, g])
                        if postamble is not None:
                            postamble(4 * g)
                        for j in range(4):
                            k = 4 * g + j
                            ksl = bass.ts(k, 128)
                            jsl = bass.ts(j, 512)
                            first, last = k == 0, k == NKT - 1
                            for ps_t, h in zip(ps, heads):
                                w = wq_sb[h] if h < HPC else wt[h]
                                nc.tensor.matmul(ps_t[:], w[:, ksl],
                                                 ht_t[:, jsl],
                                                 start=first, stop=last)
                    return ps

                def strip0_preamble(k):
                    # the very first matmul only needs wq0's chunk; the rest
                    # of each weight-chunk group queues behind the ht tile
                    if k % 4 == 0:
                        csl = bass.ds(128 * k, 512)
                        nc.sync.dma_start(wq_sb[0][:, csl], wq_d[0][:, csl])

                def strip0_postamble(k):
                    if k % 4 == 0:
                        csl = bass.ds(128 * k, 512)
                        for h in range(1, HPC):
                            nc.sync.dma_start(wq_sb[h][:, csl],
                                              wq_d[h][:, csl])
                        nc.sync.dma_start(wk_sb[:, csl], wk_d[:, csl])
                        nc.sync.dma_start(wv_sb[:, csl], wv_d[:, csl])


                def table_chunk(s):
                    # rope-table chunk for strip s, just before its RoPE
                    csl = bass.ts(s, 512)
                    nc.sync.dma_start(cos_sb[:, csl], cos_d[:, csl])
                    nc.sync.dma_start(sin_sb[:, csl], sin_d[:, csl])
                    if s == 0:
                        nc.sync.dma_start(ident_sb[:], ident_d[:])
                    if s == 1:
                        for m in range(n_masks):
                            nc.sync.dma_start(mask_sb[m][:], masks_d[m])

                for s in range(NSTRIP - 1):
                    ps = proj_round(s, [0, 1, 2, 3, 4, 5],
                                    preamble=strip0_preamble if s == 0 else None,
                                    postamble=strip0_postamble if s == 0 else None)
                    table_chunk(s)
                    if s >= 1:
                        # wo is only needed in phase 2; trickle it in
                        nc.sync.dma_start(wo_sb[s - 1][:], wo_d[s - 1])
                    v_pipeline(s, ps[5])
                    for h in range(HPC + 1):
                        rope(s, h, ps[h])

                # Last strip in two 3-bank rounds (hT re-streamed): round A's
                # banks drain during round B's matmuls, so phase 2's PSUM
                # pools don't stall on the phase-1 epilogue.
                s = NSTRIP - 1
                ps_a = proj_round(s, [0, 1, 4])
                table_chunk(s)
                nc.sync.dma_start(wo_sb[s - 1][:], wo_d[s - 1])
                for h in (0, 1, 4):
                    rope(s, h, ps_a[(0, 1, 4).index(h)])
                ps_b = proj_round(s, [5, 2, 3])
                nc.sync.dma_start(wo_sb[s][:], wo_d[s])
                v_pipeline(s, ps_b[0])
                for h in (2, 3):
                    rope(s, h, ps_b[(5, 2, 3).index(h)])

            # ---------------- phase 2: attention + out proj --------------
            with (
                tc.tile_pool(name="attn_sbuf", bufs=8) as asp,
                tc.tile_pool(name="attn_small", bufs=4) as asmall,
                tc.tile_pool(name="score_psum", bufs=3, space="PSUM") as spp,
                tc.tile_pool(name="oaug_psum", bufs=2, space="PSUM") as opp,
                tc.tile_pool(name="oproj_psum", bufs=3, space="PSUM") as prp,
            ):

                def oproj(i, at_list):
                    isl = bass.ts(i, 128)
                    for ns in range(NOUT):
                        po = prp.tile([128, 512], F32, tag="oproj",
                                      name=f"po{i}_{ns}")
                        for h in range(HPC):
                            nc.tensor.matmul(po[:], at_list[h][:],
                                             wo_sb[h][:, bass.ts(ns, 512)],
                                             start=(h == 0), stop=(h == HPC - 1))
                        po_sb = asp.tile([128, 512], F32, tag="posb", bufs=4,
                                         name=f"posb{i}_{ns}")
                        if ns % 2 == 0:
                            nc.vector.tensor_copy(po_sb[:], po[:])
                        else:
                            nc.scalar.copy(po_sb[:], po[:])
                        nc.sync.dma_start(out_d[isl, bass.ts(ns, 512)],
                                            po_sb[:])

                prev_at = None
                for i in range(NT):
                    at_sb = []
                    njobs = len(jobs[i])
                    for hp in range(2):
                        ps_o = [opp.tile([128, D + 1], F32, tag="oaug",
                                         name=f"pso{i}_{2 * hp + m}")
                                for m in range(2)]
                        # j-tiles in pairs: two score matmuls fill one
                        # [128,512] PSUM bank, one exp covers both, then the
                        # four PV matmuls consume quarter slices
                        jl = jobs[i]
                        for p0 in range(0, njobs, 2):
                            pair = jl[p0 : p0 + 2]
                            w = 256 * len(pair)
                            ps_s = spp.tile([128, 512], F32, tag="score",
                                            name=f"pss{i}_{hp}_{p0}")
                            for q, (j, mid) in enumerate(pair):
                                nc.tensor.matmul(
                                    ps_s[:, bass.ts(q, 256)],
                                    kt_sb[:, bass.ts(j, 128)],
                                    qt_sb[hp][:, bass.ts(i, 256)],
                                    start=True, stop=True)
                            se = asp.tile([128, 512], BF16, tag="sexp",
                                          name=f"se{i}_{hp}_{p0}")
                            nc.scalar.activation(
                                se[:, 0:w], ps_s[:, 0:w],
                                mybir.ActivationFunctionType.Exp,
                                bias=0.0, scale=float(SCALE))
                            for q, (j, mid) in enumerate(pair):
                                if mid is not None:
                                    nc.gpsimd.tensor_mul(
                                        se[:, bass.ts(q, 256)],
                                        se[:, bass.ts(q, 256)],
                                        mask_sb[mid][:])
                            for q, (j, mid) in enumerate(pair):
                                jj = p0 + q
                                for m in range(2):
                                    nc.tensor.matmul(
                                        ps_o[m][:],
                                        se[:, bass.ds(256 * q + 128 * m, 128)],
                                        vaug_sb[j][:],
                                        start=(jj == 0),
                                        stop=(jj == njobs - 1))
                        for m in range(2):
                            h = 2 * hp + m
                            recip = asmall.tile([128, 1], F32, tag="recip",
                                                name=f"rc{i}_{h}")
                            nc.vector.reciprocal(recip[:],
                                                 ps_o[m][:, D : D + 1])
                            a_n = asp.tile([128, 128], BF16, tag="anorm",
                                           name=f"an{i}_{h}")
                            nc.vector.tensor_scalar_mul(a_n[:],
                                                        ps_o[m][:, 0:D],
                                                        recip[:])
                            at_p = spp.tile([128, 128], BF16, tag="score",
                                            name=f"atp{i}_{h}")
                            nc.tensor.transpose(at_p[:], a_n[:], ident_sb[:])
                            at = asp.tile([128, 128], BF16, tag="at",
                                          bufs=10, name=f"at{i}_{h}")
                            nc.vector.tensor_copy(at[:], at_p[:])
                            at_sb.append(at)

                    if prev_at is not None:
                        oproj(i - 1, prev_at)
                    prev_at = at_sb
                oproj(NT - 1, prev_at)

    nc.compile()
    return nc


def _get_nc(cu_seqlens):
    key = np.asarray(cu_seqlens).tobytes()
    if key not in _cache:
        jobs, masks_np, cos_t, sin_s, ident = _host_prep(cu_seqlens)
        nc = _build(jobs, masks_np.shape[0])
        _cache[key] = (nc, masks_np, cos_t, sin_s, ident)
    return _cache[key]


def kernel(hidden_states, Wq, Wk, Wv, Wo, cu_seqlens):
    hidden_states = np.asarray(hidden_states)
    Wq, Wk, Wv, Wo = (np.asarray(a) for a in (Wq, Wk, Wv, Wo))
    cu_seqlens = np.asarray(cu_seqlens)
    nc, masks_np, cos_t, sin_s, ident = _get_nc(cu_seqlens)

    ht = np.ascontiguousarray(hidden_states.T).astype(ml_dtypes.bfloat16)
    # tile for contiguous DMA: [NSTRIP, NKT//4, 128, 2048] — each DMA
    # carries 4 hidden k-tiles side by side in the free dim
    ht_tiled = np.ascontiguousarray(
        ht.reshape(NKT // 4, 4, 128, NSTRIP, 512).transpose(3, 0, 2, 1, 4)
    ).reshape(NSTRIP, NKT // 4, 128, 2048)

    in_maps = []
    for c in range(N_CORES):
        wq_c = Wq[:, QD * c : QD * (c + 1)].astype(ml_dtypes.bfloat16)
        # [HPC, 128, HID]: lhsT tiles, free dim = 32 hidden k-tiles side by side
        wq_t = np.ascontiguousarray(
            wq_c.reshape(NKT, 128, HPC, 128).transpose(2, 1, 0, 3)
        ).reshape(HPC, 128, HID)
        wk_c = Wk[:, D * c : D * (c + 1)].astype(ml_dtypes.bfloat16)
        wk_t = np.ascontiguousarray(
            wk_c.reshape(NKT, 128, 128).transpose(1, 0, 2)).reshape(128, HID)
        wv_c = Wv[:, D * c : D * (c + 1)].astype(ml_dtypes.bfloat16)
        wv_t = np.ascontiguousarray(
            wv_c.reshape(NKT, 128, 128).transpose(1, 0, 2)).reshape(128, HID)
        wo_c = np.ascontiguousarray(
            Wo[QD * c : QD * (c + 1), :].astype(ml_dtypes.bfloat16)
        ).reshape(HPC, 128, HID)
        in_maps.append({
            "ht": ht_tiled, "wq": wq_t, "wk": wk_t, "wv": wv_t, "wo": wo_c,
            "cos_t": cos_t, "sin_s": sin_s, "ident": ident,
            "masks": masks_np,
        })

    res = bass_utils.run_bass_kernel_spmd(nc, in_maps,
                                          core_ids=list(range(N_CORES)))
    out = res.results[0]["out"].astype(np.float64)
    for c in range(1, N_CORES):
        out += res.results[c]["out"]
    return out.astype(np.float32)



# revision 2
# speedup vs baseline: 1.0005x; 1.0005x over previous
"""Trainium2 Bass kernel for Mistral-style sliding-window GQA attention.

v2b: single merged pipeline. Projections for strip s+1 are emitted as
per-head PSUM accumulation chains (2-bank rotation) and the attention +
out-projection work for the rows of strip s is woven between the chain
groups, so projection matmuls fill every dependency stall of the
attention datapath (exp/mask on ACT/Pool, softmax-normalize on DVE) and
vice versa. V is projected directly in [token, dim] layout (no PE
transposes for V). Output partials are written bf16.

Sharding: tensor-parallel over heads across 8 cores. Core c owns Q heads
[4c, 4c+4) and KV head c. Wq/Wk/Wv column-sharded, Wo row-sharded; each
core emits a partial [2048, 4096] output which the host sums.
"""

import numpy as np
import ml_dtypes

import concourse.bass as bass
import concourse.tile as tile
from concourse import bacc, mybir
from concourse import bass_utils

# ---- problem constants -------------------------------------------------
T = 2048
HID = 4096
NUM_HEADS = 32
NUM_KV_HEADS = 8
D = 128  # head dim
WINDOW = 512
ROPE_THETA = 10000.0
N_CORES = 8
HPC = NUM_HEADS // N_CORES  # 4 q heads per core
QD = HPC * D  # 512 q-proj cols per core

NT = T // 128  # 16 token tiles
NKT = HID // 128  # 32 hidden k-tiles
NSTRIP = T // 512  # 4 token strips of 512
NOUT = HID // 512  # 8 output column slices

F32 = mybir.dt.float32
BF16 = mybir.dt.bfloat16
SCALE = 1.0 / np.sqrt(D)

_cache = {}


def _host_prep(cu_seqlens):
    """Everything derived from cu_seqlens: positions, rope tables,
    per-tile job list and mask tiles (ST layout [k, q], head-pair
    duplicated to [128, 256])."""
    cu = np.asarray(cu_seqlens, dtype=np.int64)
    tok = np.arange(T)
    seg = np.searchsorted(cu[1:], tok, side="right")
    pos = tok - cu[np.minimum(seg, len(cu) - 1)]

    same = seg[:, None] == seg[None, :]
    causal = pos[None, :] <= pos[:, None]
    win = pos[None, :] >= pos[:, None] - (WINDOW - 1)
    allowed = same & causal & win  # [q, k]

    jobs = []  # jobs[i] = [(j, mask_id | None), ...]
    masks = []
    mask_index = {}
    for i in range(NT):
        row = []
        for j in range(NT):
            blk = allowed[128 * i : 128 * (i + 1), 128 * j : 128 * (j + 1)]
            if not blk.any():
                continue
            if blk.all():
                row.append((j, None))
            else:
                key = blk.tobytes()
                if key not in mask_index:
                    mask_index[key] = len(masks)
                    masks.append(blk.T.astype(np.float32))  # ST layout
                row.append((j, mask_index[key]))
        jobs.append(row)
    if not masks:
        masks.append(np.ones((128, 128), np.float32))
    masks_np = np.stack(masks).astype(ml_dtypes.bfloat16)

    inv = 1.0 / (ROPE_THETA ** (np.arange(0, D, 2, dtype=np.float64) / D))
    freqs = pos[:, None].astype(np.float64) * inv[None, :]  # [T, 64]
    emb = np.concatenate([freqs, freqs], axis=1)  # [T, 128]
    cos_t = np.cos(emb).T.astype(ml_dtypes.bfloat16).copy()  # [128, T]
    sin_t = np.sin(emb).T.astype(ml_dtypes.bfloat16)
    # sign-folded: rope(x)[d] = x[d]*cos[d] + x[(d+64)%128] * sin_s[d]
    sin_s = np.concatenate([-sin_t[:64], sin_t[64:]], axis=0).copy()
    ident = np.eye(128, dtype=ml_dtypes.bfloat16)

    return jobs, masks_np, cos_t, sin_s, ident


def _build(jobs, n_masks):
    """Trace the Bass/Tile program (identical on all cores)."""
    nc = bacc.Bacc("TRN2", target_bir_lowering=False, debug=False,
                   num_devices=N_CORES)

    # DRAM I/O (per-core shapes)
    ht_d = nc.dram_tensor("ht", [NSTRIP, NKT // 4, 128, 2048], BF16,
                          kind="ExternalInput").ap()
    wq_d = nc.dram_tensor("wq", [HPC, 128, HID], BF16,
                          kind="ExternalInput").ap()
    wk_d = nc.dram_tensor("wk", [128, HID], BF16, kind="ExternalInput").ap()
    wv_d = nc.dram_tensor("wv", [128, HID], BF16, kind="ExternalInput").ap()
    wo_d = nc.dram_tensor("wo", [HPC, 128, HID], BF16,
                          kind="ExternalInput").ap()
    cos_d = nc.dram_tensor("cos_t", [128, T], BF16, kind="ExternalInput").ap()
    sin_d = nc.dram_tensor("sin_s", [128, T], BF16, kind="ExternalInput").ap()
    ident_d = nc.dram_tensor("ident", [128, 128], BF16,
                             kind="ExternalInput").ap()
    masks_d = nc.dram_tensor("masks", [n_masks, 128, 128], BF16,
                             kind="ExternalInput").ap()
    out_d = nc.dram_tensor("out", [T, HID], BF16, kind="ExternalOutput").ap()

    with tile.TileContext(nc) as tc:
        with tc.tile_pool(name="persist", bufs=1) as pp:
            # resident weights / tables
            wq_sb = [pp.tile([128, HID], BF16, name=f"wq{h}") for h in range(HPC)]
            wk_sb = pp.tile([128, HID], BF16, name="wk_sb")
            wv_sb = pp.tile([128, HID], BF16, name="wv_sb")
            wo_sb = [pp.tile([128, HID], BF16, name=f"wo{h}") for h in range(HPC)]

            ident_sb = pp.tile([128, 128], BF16, name="ident_sb")
            mask_sb = [pp.tile([128, 128], BF16, name=f"mask{m}")
                       for m in range(n_masks)]
            # qt pairs: [128, 2*T]; cols [256*i + 128*m : +128] = head
            # (2*hp + m), token tile i.
            qt_sb = [pp.tile([128, 2 * T], BF16, name=f"qtp{hp}")
                     for hp in range(2)]
            kt_sb = pp.tile([128, T], BF16, name="kt_sb")
            vaug_sb = [pp.tile([128, D + 1], BF16, name=f"vaug{t}")
                       for t in range(NT)]

            qt_4d = [q.rearrange("p (i m c) -> p i m c", m=2, c=128)
                     for q in qt_sb]

            for t in range(NT):
                nc.vector.memset(vaug_sb[t][:, D : D + 1], 1.0)

            ht_tiles = {}  # (s, g) -> SBUF AP
            tb_tiles = {}  # ("cos"|"sin", s) -> SBUF AP

            with (
                tc.tile_pool(name="ht_pool", bufs=16) as htp,
                tc.tile_pool(name="rope_tmp", bufs=2) as rtp,
                tc.tile_pool(name="attn_sbuf", bufs=8) as asp,
                tc.tile_pool(name="attn_small", bufs=4) as asmall,
            ):
                def load_ht(s, g):
                    t = htp.tile([128, 2048], BF16, tag="ht",
                                 name=f"ht{s}_{g}")
                    nc.sync.dma_start(t[:], ht_d[s, g])
                    ht_tiles[(s, g)] = t

                def load_tables(s):
                    for nm, d in (("cos", cos_d), ("sin", sin_d)):
                        t = rtp.tile([128, 512], BF16, tag=nm, bufs=2,
                                     name=f"{nm}{s}")
                        nc.sync.dma_start(t[:], d[:, bass.ts(s, 512)])
                        tb_tiles[(nm, s)] = t

                def rope_copy(s, h, src):
                    """src: fp32 PSUM [128, 512]; returns raw SBUF tile."""
                    raw = rtp.tile([128, 512], F32, tag="raw", bufs=5,
                                   name=f"raw{s}_{h}")
                    nc.scalar.copy(raw[:], src[:])
                    return raw

                def rope_math(s, h, raw):
                    ssl = bass.ts(s, 512)
                    if h < HPC:
                        dst = qt_4d[h // 2][:, 4 * s : 4 * s + 4, h % 2, :]
                    else:
                        dst = kt_sb[:, ssl]
                    t1 = rtp.tile([128, 512], F32, tag="t1",
                                  name=f"t1_{s}_{h}")
                    nc.gpsimd.tensor_mul(t1[:], raw[:],
                                         tb_tiles[("cos", s)][:])
                    # rotate_half: walrus requires TT operands to share a
                    # start partition, so swap halves via shifted copies first
                    # (partition-shifted copies are legal; signs live in sin_s)
                    sw = rtp.tile([128, 512], F32, tag="sw",
                                  name=f"sw{s}_{h}")
                    nc.vector.tensor_scalar_mul(sw[0:64, :],
                                                raw[64:128, :], 1.0)
                    nc.vector.tensor_scalar_mul(sw[64:128, :],
                                                raw[0:64, :], 1.0)
                    t2 = rtp.tile([128, 512], F32, tag="t2",
                                  name=f"t2_{s}_{h}")
                    nc.vector.tensor_mul(t2[:], sw[:],
                                          tb_tiles[("sin", s)][:])
                    if h < HPC:
                        t1v = t1.rearrange("p (i c) -> p i c", c=128)
                        t2v = t2.rearrange("p (i c) -> p i c", c=128)
                    else:
                        t1v, t2v = t1[:], t2[:]
                    nc.vector.tensor_add(dst, t1v, t2v)

                # ---------------- strip 0: group-major warmup ------------
                # chain pool opened FIRST so its banks never alias strip-0's
                # (v chains / strip-1 chains must not wait on rope drains)
                cpp_ctx = tc.tile_pool(name="chain_psum", bufs=2,
                                       space="PSUM")
                cpp = cpp_ctx.__enter__()
                with tc.tile_pool(name="s0_psum", bufs=5,
                                  space="PSUM") as s0p:
                    ps0 = [s0p.tile([128, 512], F32, tag="s0",
                                    name=f"ps0_{h}") for h in range(5)]

                    def s0_w(lo, w):
                        csl = bass.ds(lo, w)
                        for h in range(HPC):
                            nc.sync.dma_start(wq_sb[h][:, csl],
                                              wq_d[h][:, csl])
                        nc.sync.dma_start(wk_sb[:, csl], wk_d[:, csl])

                    nc.sync.dma_start(wq_sb[0][:, 0:1024],
                                      wq_d[0][:, 0:1024])
                    load_ht(0, 0)
                    for h in range(1, HPC):
                        nc.sync.dma_start(wq_sb[h][:, 0:1024],
                                          wq_d[h][:, 0:1024])
                    nc.sync.dma_start(wk_sb[:, 0:1024], wk_d[:, 0:1024])
                    nc.sync.dma_start(ident_sb[:], ident_d[:])
                    for g in range(NKT // 4):
                        # ht for the next group first, then the weight
                        # lookahead chunks behind it
                        if g < 7:
                            load_ht(0, g + 1)
                        if g % 2 == 0 and g < 6:
                            s0_w(1024 * (g // 2 + 1), 1024)
                        if g < 4:
                            load_ht(1, g)
                        if g == 4:
                            load_tables(0)
                        if g == 5:
                            # v weights only needed by the v chains at the
                            # end of the strip: one full-row DMA, off the
                            # critical path
                            nc.sync.dma_start(wv_sb[:], wv_d[:])
                        ht_t = ht_tiles[(0, g)]
                        for j in range(4):
                            k = 4 * g + j
                            ksl = bass.ts(k, 128)
                            jsl = bass.ts(j, 512)
                            first, last = k == 0, k == NKT - 1
                            for h in range(5):
                                w = wq_sb[h] if h < HPC else wk_sb
                                nc.tensor.matmul(ps0[h][:], w[:, ksl],
                                                 ht_t[:, jsl],
                                                 start=first, stop=last)
                    # drain the 5 banks fast via ACT copies (k first: the
                    # first woven attention rows need kt before qt)
                    raws = {}
                    for h in (4, 0, 1, 2, 3):
                        raws[h] = rope_copy(0, h, ps0[h])

                # -------- steady-state pools (reuse strip-0's banks) ------
                with (
                    tc.tile_pool(name="score_psum", bufs=2,
                                 space="PSUM") as spp,
                    tc.tile_pool(name="oaug_psum", bufs=2,
                                 space="PSUM") as opp,
                    tc.tile_pool(name="oproj_psum", bufs=2,
                                 space="PSUM") as prp,
                ):
                    # strip 0 epilogue: V chains (DVE copies ahead of the
                    # rope DVE chains), then rope math from the raw tiles
                    def v_chain(s, tt):
                        ps = cpp.tile([128, 512], F32, tag="chain",
                                      name=f"vch{s}_{tt}")
                        for g in range(NKT // 4):
                            ht_t = ht_tiles[(s, g)]
                            for j in range(4):
                                k = 4 * g + j
                                nc.tensor.matmul(
                                    ps[:, 0:128],
                                    ht_t[:, bass.ds(512 * j + 128 * tt, 128)],
                                    wv_sb[:, bass.ts(k, 128)],
                                    start=(k == 0), stop=(k == NKT - 1))
                        nc.vector.tensor_copy(vaug_sb[4 * s + tt][:, 0:D],
                                              ps[:, 0:128])

                    for tt in range(4):
                        v_chain(0, tt)
                    for g in (4, 5, 6, 7):
                        load_ht(1, g)
                    for m in range(n_masks):
                        nc.sync.dma_start(mask_sb[m][:], masks_d[m])
                    for h in (4, 0, 1, 2, 3):
                        rope_math(0, h, raws[h])

                    # ---------------- attention row machinery -------------
                    def row_units(i, at_out):
                        """Emission units for row i; "F" marks preferred
                        filler (oproj slice) slots."""
                        hp_parts = []
                        jl = jobs[i]
                        pairs = [jl[p : p + 2]
                                 for p in range(0, len(jl), 2)]
                        np_ = len(pairs)
                        for hp in range(2):
                            ses = {}
                            state = {}

                            def mk_sc(hp, pi, pair, ses=None, state=None):
                                def f():
                                    ps_s = spp.tile(
                                        [128, 512], F32, tag="score",
                                        name=f"pss{i}_{hp}_{pi}")
                                    for q, (j, mid) in enumerate(pair):
                                        nc.tensor.matmul(
                                            ps_s[:, bass.ts(q, 256)],
                                            kt_sb[:, bass.ts(j, 128)],
                                            qt_sb[hp][:, bass.ts(i, 256)],
                                            start=True, stop=True)
                                    se = asp.tile(
                                        [128, 512], BF16, tag="sexp",
                                        bufs=5, name=f"se{i}_{hp}_{pi}")
                                    # per-job exp + DVE mask: the first PV
                                    # matmul only waits on its own job's
                                    # (shorter) exp, not the whole pair's
                                    for q, (j, mid) in enumerate(pair):
                                        qsl = bass.ts(q, 256)
                                        nc.scalar.activation(
                                            se[:, qsl], ps_s[:, qsl],
                                            mybir.ActivationFunctionType.Exp,
                                            bias=0.0, scale=float(SCALE))
                                        if mid is not None:
                                            for mh in range(2):
                                                hsl = bass.ds(
                                                    256 * q + 128 * mh, 128)
                                                nc.vector.tensor_mul(
                                                    se[:, hsl], se[:, hsl],
                                                    mask_sb[mid][:])
                                    ses[pi] = se
                                return f

                            def mk_pv(hp, pi, pair, first, last,
                                      ses=None, state=None):
                                def f():
                                    if first:
                                        state["ps_o"] = [
                                            opp.tile([128, D + 1], F32,
                                                     tag="oaug",
                                                     name=f"pso{i}_"
                                                          f"{2 * hp + m}")
                                            for m in range(2)]
                                    se = ses[pi]
                                    for q, (j, mid) in enumerate(pair):
                                        for m in range(2):
                                            nc.tensor.matmul(
                                                state["ps_o"][m][:],
                                                se[:, bass.ds(
                                                    256 * q + 128 * m, 128)],
                                                vaug_sb[j][:],
                                                start=(first and q == 0),
                                                stop=(last and
                                                      q == len(pair) - 1))
                                return f

                            def mk_norm(hp, state=None):
                                def f():
                                    for m in range(2):
                                        h = 2 * hp + m
                                        recip = asmall.tile(
                                            [128, 1], F32, tag="recip",
                                            name=f"rc{i}_{h}")
                                        nc.vector.reciprocal(
                                            recip[:],
                                            state["ps_o"][m][:, D : D + 1])
                                        a_n = asp.tile(
                                            [128, 128], BF16, tag="anorm", bufs=6,
                                            name=f"an{i}_{h}")
                                        nc.vector.tensor_scalar_mul(
                                            a_n[:],
                                            state["ps_o"][m][:, 0:D],
                                            recip[:])
                                        at_p = opp.tile(
                                            [128, 128], BF16, tag="oaug",
                                            name=f"atp{i}_{h}")
                                        nc.tensor.transpose(
                                            at_p[:], a_n[:], ident_sb[:])
                                        at = asp.tile(
                                            [128, 128], BF16, tag="at",
                                            bufs=9, name=f"at{i}_{h}")
                                        nc.vector.tensor_copy(at[:],
                                                              at_p[:])
                                        at_out.append(at)
                                return f

                            kw = dict(ses=ses, state=state)
                            if np_ == 1:
                                scs = [mk_sc(hp, 0, pairs[0], **kw), "F"]
                                pvs = [mk_pv(hp, 0, pairs[0], True, True,
                                             **kw)]
                            elif np_ == 2:
                                scs = [mk_sc(hp, 0, pairs[0], **kw),
                                       mk_sc(hp, 1, pairs[1], **kw), "F"]
                                pvs = [mk_pv(hp, 0, pairs[0], True, False,
                                             **kw),
                                       mk_pv(hp, 1, pairs[1], False, True,
                                             **kw)]
                            else:
                                scs = [mk_sc(hp, 0, pairs[0], **kw),
                                       mk_sc(hp, 1, pairs[1], **kw), "F"]
                                pvs = [mk_pv(hp, 0, pairs[0], True, False,
                                             **kw),
                                       mk_sc(hp, 2, pairs[2], **kw),
                                       mk_pv(hp, 1, pairs[1], False, False,
                                             **kw), "F",
                                       mk_pv(hp, 2, pairs[2], False, True,
                                             **kw)]
                            hp_parts.append((scs, pvs,
                                             mk_norm(hp, state=state)))
                        (sc0, pv0, n0), (sc1, pv1, n1) = hp_parts
                        return (sc0 + pv0 + sc1 + ["F", n0] + pv1 +
                                ["F", "F", n1, "F"])

                    def oproj_units(i, at_list):
                        """8 slice units: 4 matmuls + bf16 copy each; DMA
                        per posb pair. at_list read lazily."""
                        us = []
                        isl = bass.ts(i, 128)
                        posb_box = {}

                        def mk(ns):
                            def f():
                                posb = asp.tile(
                                    [128, 512], BF16, tag="posb",
                                    bufs=3, name=f"posb{i}_{ns}")
                                po = prp.tile([128, 512], F32, tag="oproj",
                                              name=f"po{i}_{ns}")
                                for h in range(HPC):
                                    nc.tensor.matmul(
                                        po[:], at_list[h][:],
                                        wo_sb[h][:, bass.ts(ns, 512)],
                                        start=(h == 0), stop=(h == HPC - 1))
                                if ns % 2 == 0:
                                    nc.vector.tensor_copy(posb[:], po[:])
                                else:
                                    nc.scalar.copy(posb[:], po[:])
                                nc.sync.dma_start(
                                    out_d[isl, bass.ts(ns, 512)], posb[:])
                            return f

                        for ns in range(NOUT):
                            us.append(mk(ns))
                        return us

                    def build_row_stream(i, prev_i, prev_at):
                        at_out = []
                        units = row_units(i, at_out)
                        fillers = (oproj_units(prev_i, prev_at)
                                   if prev_at is not None else [])
                        out = []
                        fi = 0
                        for u in units:
                            if u == "F":
                                if fi < len(fillers):
                                    out.append(fillers[fi])
                                    fi += 1
                            else:
                                out.append(u)
                        out.extend(fillers[fi:])
                        return out, at_out

                    # ---------------- steady state: strips 1..3 ----------
                    class Stream:
                        def __init__(self):
                            self.q = []
                            self.pos = 0

                        def add(self, items):
                            self.q.extend(items)

                        def emit_to(self, frac):
                            target = int(frac * len(self.q) + 1e-9)
                            while self.pos < min(target, len(self.q)):
                                self.q[self.pos]()
                                self.pos += 1

                        def drain(self):
                            self.emit_to(1.0)

                    def qk_chain(s, h, hook):
                        ps = cpp.tile([128, 512], F32, tag="chain",
                                      name=f"ch{s}_{h}")
                        w = wq_sb[h] if h < HPC else wk_sb
                        for g in range(NKT // 4):
                            ht_t = ht_tiles[(s, g)]
                            for j in range(4):
                                k = 4 * g + j
                                nc.tensor.matmul(ps[:],
                                                 w[:, bass.ts(k, 128)],
                                                 ht_t[:, bass.ts(j, 512)],
                                                 start=(k == 0),
                                                 stop=(k == NKT - 1))
                            hook(853)
                        rope_math(s, h, rope_copy(s, h, ps))

                    prev_at = None
                    prev_i = None
                    WSTART = 2  # chains before weaving starts, per strip
                    for s in range(1, NSTRIP):
                        stream = Stream()
                        for r in range(4):
                            i = 4 * (s - 1) + r
                            rs, at_out = build_row_stream(i, prev_i,
                                                          prev_at)
                            stream.add(rs)
                            prev_at = at_out
                            prev_i = i
                        # region cost for pacing (ci >= WSTART)
                        region = (5 - WSTART) * 8 * 853 + 4 * 8 * 213
                        cum = [0]

                        def hook(cost, stream=stream, cum=cum,
                                 region=region):
                            cum[0] += cost
                            stream.emit_to(cum[0] / region)

                        nohook = lambda cost: None
                        load_tables(s)
                        ci = 0
                        for h in (0, 1, 2, 3, 4):
                            if s == 1 and ci < HPC:
                                nc.sync.dma_start(wo_sb[ci][:], wo_d[ci])
                            if s < NSTRIP - 1 and ci < 4:
                                load_ht(s + 1, ci)
                            qk_chain(s, h, hook if ci >= WSTART else nohook)
                            ci += 1
                        for tt in range(4):
                            ps = cpp.tile([128, 512], F32, tag="chain",
                                          name=f"vch{s}_{tt}")
                            for g in range(NKT // 4):
                                ht_t = ht_tiles[(s, g)]
                                for j in range(4):
                                    k = 4 * g + j
                                    nc.tensor.matmul(
                                        ps[:, 0:128],
                                        ht_t[:, bass.ds(512 * j + 128 * tt,
                                                        128)],
                                        wv_sb[:, bass.ts(k, 128)],
                                        start=(k == 0),
                                        stop=(k == NKT - 1))
                                hook(213)
                            nc.vector.tensor_copy(
                                vaug_sb[4 * s + tt][:, 0:D], ps[:, 0:128])
                            ci += 1
                        if s < NSTRIP - 1:
                            for g in (4, 5, 6, 7):
                                load_ht(s + 1, g)
                        stream.drain()

                    # ---------------- tail: rows 12..15 -------------------
                    tail = Stream()
                    for i in range(4 * (NSTRIP - 1), NT):
                        rs, at_out = build_row_stream(i, prev_i, prev_at)
                        tail.add(rs)
                        prev_at = at_out
                        prev_i = i
                    tail.add(oproj_units(prev_i, prev_at))
                    tail.drain()
                cpp_ctx.__exit__(None, None, None)

    nc.compile()
    return nc


def _get_nc(cu_seqlens):
    key = np.asarray(cu_seqlens).tobytes()
    if key not in _cache:
        jobs, masks_np, cos_t, sin_s, ident = _host_prep(cu_seqlens)
        nc = _build(jobs, masks_np.shape[0])
        _cache[key] = (nc, masks_np, cos_t, sin_s, ident)
    return _cache[key]


def kernel(hidden_states, Wq, Wk, Wv, Wo, cu_seqlens):
    hidden_states = np.asarray(hidden_states)
    Wq, Wk, Wv, Wo = (np.asarray(a) for a in (Wq, Wk, Wv, Wo))
    cu_seqlens = np.asarray(cu_seqlens)
    nc, masks_np, cos_t, sin_s, ident = _get_nc(cu_seqlens)

    ht = np.ascontiguousarray(hidden_states.T).astype(ml_dtypes.bfloat16)
    # tile for contiguous DMA: [NSTRIP, NKT//4, 128, 2048] — each DMA
    # carries 4 hidden k-tiles side by side in the free dim
    ht_tiled = np.ascontiguousarray(
        ht.reshape(NKT // 4, 4, 128, NSTRIP, 512).transpose(3, 0, 2, 1, 4)
    ).reshape(NSTRIP, NKT // 4, 128, 2048)

    in_maps = []
    for c in range(N_CORES):
        wq_c = Wq[:, QD * c : QD * (c + 1)].astype(ml_dtypes.bfloat16)
        # [HPC, 128, HID]: lhsT tiles, free dim = 32 hidden k-tiles side by side
        wq_t = np.ascontiguousarray(
            wq_c.reshape(NKT, 128, HPC, 128).transpose(2, 1, 0, 3)
        ).reshape(HPC, 128, HID)
        wk_c = Wk[:, D * c : D * (c + 1)].astype(ml_dtypes.bfloat16)
        wk_t = np.ascontiguousarray(
            wk_c.reshape(NKT, 128, 128).transpose(1, 0, 2)).reshape(128, HID)
        wv_c = Wv[:, D * c : D * (c + 1)].astype(ml_dtypes.bfloat16)
        wv_t = np.ascontiguousarray(
            wv_c.reshape(NKT, 128, 128).transpose(1, 0, 2)).reshape(128, HID)
        wo_c = np.ascontiguousarray(
            Wo[QD * c : QD * (c + 1), :].astype(ml_dtypes.bfloat16)
        ).reshape(HPC, 128, HID)
        in_maps.append({
            "ht": ht_tiled, "wq": wq_t, "wk": wk_t, "wv": wv_t, "wo": wo_c,
            "cos_t": cos_t, "sin_s": sin_s, "ident": ident,
            "masks": masks_np,
        })

    res = bass_utils.run_bass_kernel_spmd(nc, in_maps,
                                          core_ids=list(range(N_CORES)))
    out = res.results[0]["out"].astype(np.float64)
    for c in range(1, N_CORES):
        out += res.results[c]["out"].astype(np.float64)
    return out.astype(np.float32)
